# revision 1
# baseline (speedup 1.0000x reference)
"""PointPillarScatter on 8 NeuronCores.

Full inputs -> full (B, C, NX, NY) float32 output.

Sharding: core k handles (sample b = k//2, output-x half h = k%2); each core
produces out[b, :, h*216:(h+1)*216, :] (the flip along x is baked into the
host-built scatter offsets).

Per-core device pipeline, two phases:

  Phase 1 (sparse scatter, ~6k rows/core):
    The ~6k real pillar rows are DMA'd densely into SBUF and scattered by
    dma_scatter_add into a runtime-pre-zeroed DRAM staging canvas.  Staging is
    laid out partition-major: partition p owns 838 consecutive rows (837 canvas
    positions {i : i % 128 == p} ordered by i // 128, plus 1 dump row for the
    padding slots), so the offsets bake in both the scatter and the
    transpose-friendly permutation, and int16 offsets stay in range per
    32-partition region.

  Phase 2 (dense stream, memory-bound):
    Per chunk of 24 output-x rows: one big contiguous DMA pulls the staging
    slice into SBUF as [128 pos-in-block, 93 blocks, 64 ch]; PE transposes
    pairs of 128-position blocks through an identity ([128,128] -> PSUM);
    DVE/ACT copy PSUM into the [64 ch, 11904 pos] out tile; one 3 MB DMA
    writes the (C, X, Y) canvas slice.
"""

import sys

sys.path.insert(0, "/opt/trn_rl_repo")

import numpy as np

import concourse.bacc as bacc
import concourse.mybir as mybir
from concourse.bass_utils import run_bass_kernel_spmd
from concourse.masks import make_identity
from concourse.tile import TileContext

C = 64
NX = 432
NY = 496
B = 4
NCORES = 8
XH = NX // 2            # 216 x-rows per core
M = XH * NY             # 107136 positions per core
P = 128
JPP = M // P            # 837 rows of 128 positions per partition
XCHUNK = 8
NCHUNK = XH // XCHUNK   # 27
MC = XCHUNK * NY        # 3968 positions per chunk
JBLK = MC // P          # 31 blocks of 128 positions
CSPLITS = [0, 2, 9, 18, 27]             # chunk ranges per staging tensor
NSPLIT = len(CSPLITS) - 1
JS = [(CSPLITS[i + 1] - CSPLITS[i]) * JBLK for i in range(NSPLIT)]   # rows/partition
RPS = [j + 1 for j in JS]               # +1 dump row
NREG = 2                # int16 offsets cover 64 partitions x <=218 rows
PREG = P // NREG        # 64 partitions per region

_CACHE = {}
LAST_RESULTS = None


def _build_program(jr):
    nslot = P * jr          # scatter slots per segment (padded, fixed count)
    NSEG = NSPLIT * NREG    # (j-split, region)
    nc = bacc.Bacc(None, target_bir_lowering=False)
    feats = nc.dram_tensor("feats", [NSEG * nslot, C], mybir.dt.float32, kind="ExternalInput")
    sidx = nc.dram_tensor("sidx", [P, NSEG * nslot // 16], mybir.dt.int16, kind="ExternalInput")
    sts = [
        nc.dram_tensor(f"st{i}", [P * RPS[i], C], mybir.dt.float32, kind="ExternalOutput")
        for i in range(NSPLIT)
    ]
    out = nc.dram_tensor("out", [C, XH, NY], mybir.dt.float32, kind="ExternalOutput")

    views = [sts[i][:].rearrange("(pt j) c -> pt j c", j=RPS[i]) for i in range(NSPLIT)]

    with TileContext(nc) as tc:
        with (
            tc.tile_pool(name="scat", bufs=2) as scatp,
            tc.tile_pool(name="sidxp", bufs=2) as sidxp,
            tc.tile_pool(name="const", bufs=1) as constp,
            tc.tile_pool(name="gather", bufs=6) as gatherp,
            tc.tile_pool(name="outp", bufs=4) as outp,
            tc.tile_pool(name="psum", bufs=4, space="PSUM") as psump,
            tc.tile_pool(name="psums", bufs=2, space="PSUM") as psumsp,
        ):
            for seg in range(NSEG):
                sp, r = divmod(seg, NREG)
                regrows = PREG * RPS[sp]
                ft = scatp.tile([P, jr, C], mybir.dt.float32, tag="ft")
                nc.scalar.dma_start(ft[:], feats[seg * nslot:(seg + 1) * nslot, :].rearrange("(p j) c -> p j c", j=jr))
                it = sidxp.tile([P, nslot // 16], mybir.dt.int16, tag="it")
                nc.scalar.dma_start(it[:], sidx[:, seg * (nslot // 16):(seg + 1) * (nslot // 16)])
                nc.gpsimd.dma_scatter_add(
                    out_ap=sts[sp][r * regrows:(r + 1) * regrows, :],
                    in_ap=ft[:],
                    idxs_ap=it[:],
                    num_idxs=nslot,
                    num_idxs_reg=nslot,
                    elem_size=C,
                    single_packet=False,
                )

            ident = constp.tile([P, P], mybir.dt.float32)
            make_identity(nc, ident[:])

            for ci in range(NCHUNK):
                sp = next(i for i in range(NSPLIT) if CSPLITS[i] <= ci < CSPLITS[i + 1])
                cl = ci - CSPLITS[sp]
                src = views[sp][:, cl * JBLK:(cl + 1) * JBLK, :]
                gt = gatherp.tile([P, JBLK * C], mybir.dt.float32, tag="gt")
                nc.scalar.dma_start(gt[:].rearrange("p (j c) -> p j c", c=C), src)

                ot = outp.tile([C, MC], mybir.dt.float32, tag="ot")
                npairs = JBLK // 2
                nquads = (npairs + 3) // 4
                for q in range(nquads):
                    np_q = min(4, npairs - q * 4)
                    pt = psump.tile([P, 512], mybir.dt.float32, tag="pt")
                    for m in range(np_q):
                        k = q * 4 + m
                        nc.tensor.transpose(pt[:, m * P:(m + 1) * P], gt[:, k * P:(k + 1) * P], ident[:])
                    base = q * 4 * 2 * P
                    dst = ot[:, base:base + np_q * 2 * P].rearrange("c (n two x) -> c n two x", two=2, x=P)
                    src_ps = pt[:, :np_q * P]
                    nc.vector.tensor_copy(dst[:, :, 0, :], src_ps[0:C, :].rearrange("c (n x) -> c n x", x=P))
                    nc.scalar.copy(dst[:, :, 1, :], src_ps[C:P, :].rearrange("c (n x) -> c n x", x=P))
                j = JBLK - 1
                pt = psumsp.tile([P, P], mybir.dt.float32, tag="pts")
                nc.tensor.transpose(pt[0:C, :], gt[:, j * C:(j + 1) * C], ident[:])
                nc.vector.tensor_copy(ot[:, j * P:(j + 1) * P], pt[0:C, :])
                nc.sync.dma_start(out[:, ci * XCHUNK:(ci + 1) * XCHUNK, :], ot[:].rearrange("c (x y) -> c x y", y=NY))

    nc.finalize()
    return nc


def _prep_in_maps(feats_full, batch_indices, sample_indices):
    x = batch_indices[:, 2].astype(np.int64)
    y = batch_indices[:, 1].astype(np.int64)
    sm = sample_indices.astype(np.int64)
    xo = (NX - 1) - x
    h = xo // XH
    xl = xo % XH
    pos = xl * NY + y
    core = sm * 2 + h

    pp = pos % P            # partition
    jj = pos // P           # row within partition
    reg = pp // PREG

    jbounds = np.array([c * JBLK for c in CSPLITS])
    sp = np.searchsorted(jbounds, jj, side="right") - 1     # which staging tensor
    rp_arr = np.array(RPS)[sp]
    jloc = jj - jbounds[sp]
    seg = sp * NREG + reg
    local = (pp % PREG) * rp_arr + jloc                     # int16-safe

    NSEG = NSPLIT * NREG
    maxn = 0
    for k in range(NCORES):
        for g in range(NSEG):
            maxn = max(maxn, int(np.sum((core == k) & (seg == g))))
    jr = -(-(maxn + 1) // P) + 1     # ceil to 128 slots + 1 spare column

    nslot = P * jr
    in_maps = []
    for k in range(NCORES):
        feats_arr = np.zeros((NSEG * nslot, C), np.float32)
        idx_arr = np.full((16, NSEG * nslot // 16), 0, np.int16)
        for g in range(NSEG):
            sp_g = g // NREG
            rows = np.nonzero((core == k) & (seg == g))[0]
            loc = local[rows]
            order = np.argsort(loc)
            rows = rows[order]
            loc = loc[order]
            n = rows.size
            assert n <= nslot
            slots = np.arange(nslot)
            vals = np.full(nslot, 0, np.int16)
            vals[:n] = loc.astype(np.int16)
            vals[n:] = ((slots[n:] % P) % PREG) * RPS[sp_g] + JS[sp_g]  # dump row
            d = (slots[:n] % P) * jr + slots[:n] // P
            feats_arr[g * nslot + d] = feats_full[rows]
            idx_arr[:, g * (nslot // 16):(g + 1) * (nslot // 16)] = vals.reshape(nslot // 16, 16).T
        in_maps.append({"feats": feats_arr, "sidx": np.ascontiguousarray(np.tile(idx_arr, (8, 1)))})
    return in_maps, jr


def kernel(batch_pillar_features, batch_indices, sample_indices, batch_size):
    global LAST_RESULTS
    feats_full = np.asarray(batch_pillar_features, np.float32)
    batch_indices = np.asarray(batch_indices)
    sample_indices = np.asarray(sample_indices)
    bs = int(batch_size)
    assert bs == B and feats_full.shape[1] == C

    in_maps, jr = _prep_in_maps(feats_full, batch_indices, sample_indices)
    if _CACHE.get("jr") != jr:
        _CACHE["nc"] = _build_program(jr)
        _CACHE["jr"] = jr
    nc = _CACHE["nc"]

    res = run_bass_kernel_spmd(nc, in_maps, core_ids=list(range(NCORES)))
    LAST_RESULTS = res

    full = np.empty((B, C, NX, NY), np.float32)
    for k in range(NCORES):
        b, hh = k // 2, k % 2
        full[b, :, hh * XH:(hh + 1) * XH, :] = res.results[k]["out"]
    return full



# revision 3
# speedup vs baseline: 1.8280x; 1.8280x over previous
"""PointPillarScatter on 8 NeuronCores — v2 (SBUF staging, bf16 pipeline).

Full inputs -> full (B, C, NX, NY) float32 output.

Sharding: core k handles (sample b = k//2, output-x half h = k%2); each core
produces out[b, :, h*216:(h+1)*216, :] (flip along x baked into host-built
scatter positions).

Per-core device pipeline (all bf16 on device; host up-casts the output):
  9 groups of 24 output-x rows (11904 positions, padded to 96 blocks of 128).
  Per group:
    1. DMA the group's pillar rows (dense, host-packed) + int16 position ids.
    2. DVE memset of a [128, 2*3072] bf16 staging tile.
    3. gpsimd dma_scatter_add in SBUF-destination parity-split mode
       (sbuf_tokens_per_rank=128): token with idx=pos lands in partition
       pos%128 of parity tile (pos//128)%2 at column (pos//256). Pad tokens
       (zero rows) are dumped into padding block 94.
    4. PE transposes (bf16, 1 cyc/row): staging cols (a, a+24) pair ->
       PSUM [128,128]; psum rows 0:64 = block 2a (first position half),
       rows 64:128 = block 2a+48 (second half).
    5. One [128, 4x128] copy per 4-transpose PSUM tile into the [128, 6144]
       out tile whose partition halves are the two position halves
       (DVE for even blocks, ACT for odd blocks).
    6. Two contiguous DMAs to the bf16 DRAM out slice.
"""

import sys

sys.path.insert(0, "/opt/trn_rl_repo")

import ml_dtypes
import numpy as np

import concourse.bacc as bacc
import concourse.mybir as mybir
from concourse.bass_utils import run_bass_kernel_spmd
from concourse.masks import make_identity
from concourse.tile import TileContext

C = 64
NX = 432
NY = 496
B = 4
NCORES = 8
XH = NX // 2            # 216 x-rows per core
P = 128
XG = 24                 # x-rows per group
NG = XH // XG           # 9 groups
MG = XG * NY            # 11904 positions per group
JG = 96                 # blocks per group, padded (93 real)
HC = JG // 2            # 48 columns per parity tile
HB = HC * C             # 3072 bf16 elems per partition per parity tile
OHALF = HC * P          # 6144 positions per partition-half of the out tile
DUMP = 94 * P           # pad tokens -> padding block 94 (never DMA'd out)

_CACHE = {}
LAST_RESULTS = None


def _build_program(jr):
    ntok = P * jr           # tokens per group (padded)
    nc = bacc.Bacc(None, target_bir_lowering=False)
    feats = nc.dram_tensor("feats", [NG * ntok, C], mybir.dt.bfloat16, kind="ExternalInput")
    sidx = nc.dram_tensor("sidx", [P, NG * ntok // 16], mybir.dt.int16, kind="ExternalInput")
    out = nc.dram_tensor("out", [C, XH * NY], mybir.dt.bfloat16, kind="ExternalOutput")

    with TileContext(nc) as tc:
        with (
            tc.tile_pool(name="featp", bufs=3) as featp,
            tc.tile_pool(name="idxp", bufs=3) as idxp,
            tc.tile_pool(name="stp", bufs=2) as stp,
            tc.tile_pool(name="outp", bufs=2) as outp,
            tc.tile_pool(name="const", bufs=1) as constp,
            tc.tile_pool(name="psum", bufs=8, space="PSUM") as psump,
        ):
            ident = constp.tile([P, P], mybir.dt.bfloat16)
            make_identity(nc, ident[:])

            for g in range(NG):
                ft = featp.tile([P, jr, C], mybir.dt.bfloat16, tag="ft")
                nc.sync.dma_start(
                    ft[:], feats[g * ntok:(g + 1) * ntok, :].rearrange("(p j) c -> p j c", j=jr)
                )
                it = idxp.tile([P, ntok // 16], mybir.dt.int16, tag="it")
                nc.sync.dma_start(it[:], sidx[:, g * (ntok // 16):(g + 1) * (ntok // 16)])

                st = stp.tile([P, 2 * HB], mybir.dt.bfloat16, tag="st")
                nc.vector.memset(st[:], 0.0)
                nc.gpsimd.dma_scatter_add(
                    out_ap=st[:, 0:HB],
                    out_ap_other=st[:, HB:2 * HB],
                    parity_reg=0,
                    in_ap=ft[:],
                    idxs_ap=it[:],
                    num_idxs=ntok,
                    num_idxs_reg=ntok,
                    elem_size=C,
                    sbuf_tokens_per_rank=P,
                    single_packet=False,
                )

                ot = outp.tile([P, OHALF], mybir.dt.bfloat16, tag="ot")
                for t in range(2):
                    # pair p = staging cols (2p, 2p+1) = blocks (2p+t, 2p+48+t)
                    stv = st[:, t * HB:(t + 1) * HB]
                    for u in range(6):
                        pt = psump.tile([P, 512], mybir.dt.bfloat16, tag="pt")
                        for m in range(4):
                            p = 4 * u + m
                            nc.tensor.transpose(
                                pt[:, m * P:(m + 1) * P], stv[:, p * P:(p + 1) * P], ident[:]
                            )
                        dv = ot[:, 1024 * u:1024 * (u + 1)].rearrange(
                            "c (m two x) -> c m two x", two=2, x=P
                        )
                        src = pt[:].rearrange("c (m x) -> c m x", x=P)
                        if t == 0:
                            nc.vector.tensor_copy(dv[:, :, 0, :], src)
                        else:
                            nc.scalar.copy(dv[:, :, 1, :], src)

                nc.sync.dma_start(out[:, g * MG:g * MG + OHALF], ot[0:C, :])
                nc.sync.dma_start(out[:, g * MG + OHALF:(g + 1) * MG], ot[C:P, 0:MG - OHALF])

    nc.finalize()
    return nc


def _prep_in_maps(feats_full, batch_indices, sample_indices):
    x = batch_indices[:, 2].astype(np.int64)
    y = batch_indices[:, 1].astype(np.int64)
    sm = sample_indices.astype(np.int64)
    xo = (NX - 1) - x               # flip along x
    h = xo // XH
    xl = xo % XH
    core = sm * 2 + h
    grp = xl // XG
    pos = (xl % XG) * NY + y        # position within group, < MG
    # Scatter slot id: block b=pos//128 maps to slot f(b) so that the two
    # blocks of one transpose pair (b, b+48) occupy adjacent staging columns
    # of the same parity tile.
    blk = pos // P
    slot = np.where(
        blk % 2 == 0,
        np.where(blk < JG // 2, 2 * blk, 2 * blk - (JG - 2)),
        np.where(blk < JG // 2, 2 * blk - 1, 2 * blk - (JG - 1)),
    )
    sid = (pos % P + P * slot).astype(np.int64)

    seg = core * NG + grp
    counts = np.bincount(seg, minlength=NCORES * NG)
    jr = -(-int(counts.max()) // P)
    ntok = P * jr

    fb = np.asarray(feats_full, np.float32).astype(ml_dtypes.bfloat16)
    in_maps = []
    for k in range(NCORES):
        fa = np.zeros((NG * ntok, C), ml_dtypes.bfloat16)
        ia = np.empty((16, NG * ntok // 16), np.int16)
        for gg in range(NG):
            rows = np.nonzero((core == k) & (grp == gg))[0]
            n = rows.size
            i = np.arange(n)
            fa[gg * ntok + (i % P) * jr + i // P] = fb[rows]
            vals = np.full(ntok, DUMP, np.int16)
            vals[:n] = sid[rows].astype(np.int16)
            ia[:, gg * (ntok // 16):(gg + 1) * (ntok // 16)] = vals.reshape(ntok // 16, 16).T
        in_maps.append({"feats": fa, "sidx": np.ascontiguousarray(np.tile(ia, (8, 1)))})
    return in_maps, jr


def kernel(batch_pillar_features, batch_indices, sample_indices, batch_size):
    global LAST_RESULTS
    feats_full = np.asarray(batch_pillar_features, np.float32)
    batch_indices = np.asarray(batch_indices)
    sample_indices = np.asarray(sample_indices)
    bs = int(batch_size)
    assert bs == B and feats_full.shape[1] == C

    in_maps, jr = _prep_in_maps(feats_full, batch_indices, sample_indices)
    if _CACHE.get("jr") != jr:
        _CACHE["nc"] = _build_program(jr)
        _CACHE["jr"] = jr
    nc = _CACHE["nc"]

    res = run_bass_kernel_spmd(nc, in_maps, core_ids=list(range(NCORES)))
    LAST_RESULTS = res

    full = np.empty((B, C, NX, NY), np.float32)
    for k in range(NCORES):
        b, hh = k // 2, k % 2
        r = np.asarray(res.results[k]["out"]).astype(np.float32).reshape(C, XH, NY)
        full[b, :, hh * XH:(hh + 1) * XH, :] = r
    return full


# revision 4
# speedup vs baseline: 2.3659x; 1.2942x over previous
"""PointPillarScatter on 8 NeuronCores — v3 (software-pipelined scatter prologue).

Same structure as v2 (SBUF parity scatter + bf16 PE transposes + partition-half
out tile), with:
  - scatter stage (dma in, memset, dma_scatter_add) runs STAGE groups ahead of
    the transpose stage in program order, so gpsimd/DVE never serialize against
    the PE stream at group boundaries;
  - full-bank [128, 1024] bf16 PSUM tiles (8 transposes per tile, one copy per
    engine per tile).
"""

import sys

sys.path.insert(0, "/opt/trn_rl_repo")

import ml_dtypes
import numpy as np

import concourse.bacc as bacc
import concourse.mybir as mybir
from concourse.bass_utils import run_bass_kernel_spmd
from concourse.masks import make_identity
from concourse.tile import TileContext

C = 64
NX = 432
NY = 496
B = 4
NCORES = 8
XH = NX // 2            # 216 x-rows per core
P = 128
XG = 24                 # x-rows per group
NG = XH // XG           # 9 groups
MG = XG * NY            # 11904 positions per group
JG = 96                 # blocks per group, padded (93 real)
HC = JG // 2            # 48 columns per parity tile
HB = HC * C             # 3072 bf16 elems per partition per parity tile
OHALF = HC * P          # 6144 positions per partition-half of the out tile
DUMP = 94 * P           # pad tokens -> padding block 94 (never DMA'd out)
STAGE = 3               # scatter stage runs this many groups ahead

_CACHE = {}
LAST_RESULTS = None


def _build_program(jr):
    ntok = P * jr           # tokens per group (padded)
    nc = bacc.Bacc(None, target_bir_lowering=False)
    feats = nc.dram_tensor("feats", [NG * ntok, C], mybir.dt.bfloat16, kind="ExternalInput")
    sidx = nc.dram_tensor("sidx", [P, NG * ntok // 16], mybir.dt.int16, kind="ExternalInput")
    out = nc.dram_tensor("out", [C, XH * NY], mybir.dt.bfloat16, kind="ExternalOutput")

    with TileContext(nc) as tc:
        with (
            tc.tile_pool(name="featp", bufs=STAGE + 1) as featp,
            tc.tile_pool(name="idxp", bufs=STAGE + 1) as idxp,
            tc.tile_pool(name="stp", bufs=STAGE + 1) as stp,
            tc.tile_pool(name="outp", bufs=2) as outp,
            tc.tile_pool(name="const", bufs=1) as constp,
            tc.tile_pool(name="psum", bufs=6, space="PSUM") as psump,
        ):
            ident = constp.tile([P, P], mybir.dt.bfloat16)
            make_identity(nc, ident[:])

            stage_tiles = {}

            def emit_scatter_stage(g):
                ft = featp.tile([P, jr, C], mybir.dt.bfloat16, tag="ft")
                nc.sync.dma_start(
                    ft[:], feats[g * ntok:(g + 1) * ntok, :].rearrange("(p j) c -> p j c", j=jr)
                )
                it = idxp.tile([P, ntok // 16], mybir.dt.int16, tag="it")
                nc.sync.dma_start(it[:], sidx[:, g * (ntok // 16):(g + 1) * (ntok // 16)])
                st = stp.tile([P, 2 * HB], mybir.dt.bfloat16, tag="st")
                nc.vector.memset(st[:], 0.0)
                nc.gpsimd.dma_scatter_add(
                    out_ap=st[:, 0:HB],
                    out_ap_other=st[:, HB:2 * HB],
                    parity_reg=0,
                    in_ap=ft[:],
                    idxs_ap=it[:],
                    num_idxs=ntok,
                    num_idxs_reg=ntok,
                    elem_size=C,
                    sbuf_tokens_per_rank=P,
                    single_packet=False,
                )
                stage_tiles[g] = st

            for g in range(min(STAGE, NG)):
                emit_scatter_stage(g)

            for g in range(NG):
                if g + STAGE < NG:
                    emit_scatter_stage(g + STAGE)
                st = stage_tiles.pop(g)
                ot = outp.tile([P, OHALF], mybir.dt.bfloat16, tag="ot")
                for t in range(2):
                    stv = st[:, t * HB:(t + 1) * HB]
                    for u in range(3):
                        pt = psump.tile([P, 1024], mybir.dt.bfloat16, tag="pt")
                        for m in range(8):
                            p = 8 * u + m
                            nc.tensor.transpose(
                                pt[:, m * P:(m + 1) * P], stv[:, p * P:(p + 1) * P], ident[:]
                            )
                        dv = ot[:, 2048 * u:2048 * (u + 1)].rearrange(
                            "c (m two x) -> c m two x", two=2, x=P
                        )
                        src = pt[:].rearrange("c (m x) -> c m x", x=P)
                        if t == 0:
                            nc.vector.tensor_copy(dv[:, :, 0, :], src)
                        else:
                            nc.scalar.copy(dv[:, :, 1, :], src)

                nc.sync.dma_start(out[:, g * MG:g * MG + OHALF], ot[0:C, :])
                nc.sync.dma_start(out[:, g * MG + OHALF:(g + 1) * MG], ot[C:P, 0:MG - OHALF])

    nc.finalize()
    return nc


def _prep_in_maps(feats_full, batch_indices, sample_indices):
    x = batch_indices[:, 2].astype(np.int64)
    y = batch_indices[:, 1].astype(np.int64)
    sm = sample_indices.astype(np.int64)
    xo = (NX - 1) - x               # flip along x
    h = xo // XH
    xl = xo % XH
    core = sm * 2 + h
    grp = xl // XG
    pos = (xl % XG) * NY + y        # position within group, < MG
    # Scatter slot id: block b=pos//128 maps to slot f(b) so that the two
    # blocks of one transpose pair (b, b+48) occupy adjacent staging columns
    # of the same parity tile.
    blk = pos // P
    slot = np.where(
        blk % 2 == 0,
        np.where(blk < JG // 2, 2 * blk, 2 * blk - (JG - 2)),
        np.where(blk < JG // 2, 2 * blk - 1, 2 * blk - (JG - 1)),
    )
    sid = (pos % P + P * slot).astype(np.int64)

    seg = core * NG + grp
    counts = np.bincount(seg, minlength=NCORES * NG)
    jr = -(-int(counts.max()) // P)
    ntok = P * jr

    fb = np.asarray(feats_full, np.float32).astype(ml_dtypes.bfloat16)
    in_maps = []
    for k in range(NCORES):
        fa = np.zeros((NG * ntok, C), ml_dtypes.bfloat16)
        ia = np.empty((16, NG * ntok // 16), np.int16)
        for gg in range(NG):
            rows = np.nonzero((core == k) & (grp == gg))[0]
            n = rows.size
            i = np.arange(n)
            fa[gg * ntok + (i % P) * jr + i // P] = fb[rows]
            vals = np.full(ntok, DUMP, np.int16)
            vals[:n] = sid[rows].astype(np.int16)
            ia[:, gg * (ntok // 16):(gg + 1) * (ntok // 16)] = vals.reshape(ntok // 16, 16).T
        in_maps.append({"feats": fa, "sidx": np.ascontiguousarray(np.tile(ia, (8, 1)))})
    return in_maps, jr


def kernel(batch_pillar_features, batch_indices, sample_indices, batch_size):
    global LAST_RESULTS
    feats_full = np.asarray(batch_pillar_features, np.float32)
    batch_indices = np.asarray(batch_indices)
    sample_indices = np.asarray(sample_indices)
    bs = int(batch_size)
    assert bs == B and feats_full.shape[1] == C

    in_maps, jr = _prep_in_maps(feats_full, batch_indices, sample_indices)
    if _CACHE.get("jr") != jr:
        _CACHE["nc"] = _build_program(jr)
        _CACHE["jr"] = jr
    nc = _CACHE["nc"]

    res = run_bass_kernel_spmd(nc, in_maps, core_ids=list(range(NCORES)))
    LAST_RESULTS = res

    full = np.empty((B, C, NX, NY), np.float32)
    for k in range(NCORES):
        b, hh = k // 2, k % 2
        r = np.asarray(res.results[k]["out"]).astype(np.float32).reshape(C, XH, NY)
        full[b, :, hh * XH:(hh + 1) * XH, :] = r
    return full


# revision 6
# speedup vs baseline: 2.3860x; 1.0085x over previous
"""PointPillarScatter on 8 NeuronCores — v5.

v4 + fixes from the v4 trace:
  - group sizes ramp up AND back down [8,16,32,40,40,32,24,16,8]: small first
    group shrinks the serial prologue, small last group shrinks the final
    write-drain tail;
  - every group gets >=1 padding block (JGP > JG strictly), so pad/dump tokens
    never CCE-add onto cells that real tokens target (concurrent read-modify-
    write adds to one SBUF cell race on HW);
  - the ~9.3us gpsimd extended-instruction library load is absorbed early by a
    dependency-free 16-token warm-up scatter issued as gpsimd's first custom
    instruction;
  - the transpose identity is uploaded from the host instead of being built
    with gpsimd affine_select, keeping the gpsimd stream free of std-lib ops.
"""

import sys

sys.path.insert(0, "/opt/trn_rl_repo")

import ml_dtypes
import numpy as np

import concourse.bacc as bacc
import concourse.mybir as mybir
from concourse.bass_utils import run_bass_kernel_spmd
from concourse.tile import TileContext

C = 64
NX = 432
NY = 496
B = 4
NCORES = 8
XH = NX // 2            # 216 x-rows per core
P = 128
XGS = [8, 16, 32, 40, 40, 32, 24, 16, 8]
assert sum(XGS) == XH and all(x % 8 == 0 for x in XGS)
NG = len(XGS)
MGS = [x * NY for x in XGS]                 # positions per group
JGS = [m // P for m in MGS]                 # real blocks per group
# padded blocks: next multiple of 4 STRICTLY greater than JG, so every group
# has at least one padding block for dump tokens
JGPS = [j + (4 - j % 4 if j % 4 else 4) for j in JGS]
HCS = [j // 2 for j in JGPS]                # columns per parity tile
GBASE = np.cumsum([0] + MGS).tolist()       # position offset of each group
STAGE = 3

_CACHE = {}
LAST_RESULTS = None


def _slot_map(jgp, blk):
    """Block -> scatter slot so transpose pairs (b, b+jgp/2) are adjacent cols."""
    half = jgp // 2
    return np.where(
        blk % 2 == 0,
        np.where(blk < half, 2 * blk, 2 * blk - (jgp - 2)),
        np.where(blk < half, 2 * blk - 1, 2 * blk - (jgp - 1)),
    )


def _dump_slot(g):
    """Slot of the first padding block."""
    jg, jgp = JGS[g], JGPS[g]
    assert jgp > jg
    b = np.array([jg])
    return int(_slot_map(jgp, b)[0])


def _build_program(jrs):
    ntoks = [P * jr for jr in jrs]
    foff = np.cumsum([0] + ntoks).tolist()
    ioff = [o // 16 for o in foff]
    nc = bacc.Bacc(None, target_bir_lowering=False)
    feats = nc.dram_tensor("feats", [foff[-1], C], mybir.dt.bfloat16, kind="ExternalInput")
    sidx = nc.dram_tensor("sidx", [P, ioff[-1]], mybir.dt.int16, kind="ExternalInput")
    idin = nc.dram_tensor("idin", [P, P], mybir.dt.bfloat16, kind="ExternalInput")
    out = nc.dram_tensor("out", [C, XH * NY], mybir.dt.bfloat16, kind="ExternalOutput")

    with TileContext(nc) as tc:
        with (
            tc.tile_pool(name="featp", bufs=STAGE + 1) as featp,
            tc.tile_pool(name="idxp", bufs=STAGE + 1) as idxp,
            tc.tile_pool(name="stp", bufs=STAGE + 1) as stp,
            tc.tile_pool(name="outp", bufs=2) as outp,
            tc.tile_pool(name="const", bufs=1) as constp,
            tc.tile_pool(name="psum", bufs=8, space="PSUM") as psump,
        ):
            ident = constp.tile([P, P], mybir.dt.bfloat16)
            nc.sync.dma_start(ident[:], idin[:])

            # Warm-up scatter: gpsimd's first custom instruction, no external
            # deps (idx tile is gpsimd-memset to 0), so the ~9us mlp-library
            # load overlaps the input DMAs / staging memsets of group 0.
            widx = constp.tile([P, 1], mybir.dt.int16)
            nc.gpsimd.memset(widx[:], 0)
            wsrc = constp.tile([P, C], mybir.dt.bfloat16)
            nc.gpsimd.memset(wsrc[:], 0.0)
            wdst = constp.tile([P, 2 * C], mybir.dt.bfloat16)
            nc.gpsimd.dma_scatter_add(
                out_ap=wdst[:, 0:C],
                out_ap_other=wdst[:, C:2 * C],
                parity_reg=0,
                in_ap=wsrc[:].rearrange("p (j c) -> p j c", c=C),
                idxs_ap=widx[:],
                num_idxs=16,
                num_idxs_reg=16,
                elem_size=C,
                sbuf_tokens_per_rank=P,
                single_packet=False,
            )

            stage_tiles = {}

            def emit_scatter_stage(g):
                jr, ntok, hb = jrs[g], ntoks[g], HCS[g] * C
                ft = featp.tile([P, jr, C], mybir.dt.bfloat16, tag="ft")
                nc.sync.dma_start(
                    ft[:], feats[foff[g]:foff[g + 1], :].rearrange("(p j) c -> p j c", j=jr)
                )
                it = idxp.tile([P, ntok // 16], mybir.dt.int16, tag="it")
                nc.sync.dma_start(it[:], sidx[:, ioff[g]:ioff[g + 1]])
                st = stp.tile([P, 2 * hb], mybir.dt.bfloat16, tag="st")
                nc.vector.memset(st[:], 0.0)
                nc.gpsimd.dma_scatter_add(
                    out_ap=st[:, 0:hb],
                    out_ap_other=st[:, hb:2 * hb],
                    parity_reg=0,
                    in_ap=ft[:],
                    idxs_ap=it[:],
                    num_idxs=ntok,
                    num_idxs_reg=ntok,
                    elem_size=C,
                    sbuf_tokens_per_rank=P,
                    single_packet=False,
                )
                stage_tiles[g] = st

            for g in range(min(STAGE, NG)):
                emit_scatter_stage(g)

            for g in range(NG):
                if g + STAGE < NG:
                    emit_scatter_stage(g + STAGE)
                st = stage_tiles.pop(g)
                hc, hb, mg = HCS[g], HCS[g] * C, MGS[g]
                ohalf = hc * P
                npairs = hc // 2
                ot = outp.tile([P, ohalf], mybir.dt.bfloat16, tag="ot")
                for t in range(2):
                    stv = st[:, t * hb:(t + 1) * hb]
                    for u in range(-(-npairs // 8)):
                        mn = min(8, npairs - 8 * u)
                        pt = psump.tile([P, mn * P], mybir.dt.bfloat16, tag="pt")
                        for m in range(mn):
                            p = 8 * u + m
                            nc.tensor.transpose(
                                pt[:, m * P:(m + 1) * P], stv[:, p * P:(p + 1) * P], ident[:]
                            )
                        dv = ot[:, 2048 * u:2048 * u + mn * 2 * P].rearrange(
                            "c (m two x) -> c m two x", two=2, x=P
                        )
                        src = pt[:].rearrange("c (m x) -> c m x", x=P)
                        if (t + u) % 2 == 0:
                            nc.vector.tensor_copy(dv[:, :, t, :], src)
                        else:
                            nc.scalar.copy(dv[:, :, t, :], src)

                gb = GBASE[g]
                nc.sync.dma_start(out[:, gb:gb + ohalf], ot[0:C, :])
                nc.sync.dma_start(out[:, gb + ohalf:gb + mg], ot[C:P, 0:mg - ohalf])

    nc.finalize()
    return nc


def _prep_in_maps(feats_full, batch_indices, sample_indices):
    x = batch_indices[:, 2].astype(np.int64)
    y = batch_indices[:, 1].astype(np.int64)
    sm = sample_indices.astype(np.int64)
    xo = (NX - 1) - x               # flip along x
    h = xo // XH
    xl = xo % XH
    core = sm * 2 + h

    xbounds = np.cumsum([0] + XGS)
    grp = np.searchsorted(xbounds, xl, side="right") - 1
    xin = xl - xbounds[grp]
    pos = xin * NY + y              # position within group
    blk = pos // P
    slot = np.empty_like(pos)
    for g in range(NG):
        msk = grp == g
        slot[msk] = _slot_map(JGPS[g], blk[msk])
    sid = pos % P + P * slot

    counts = np.zeros((NCORES, NG), np.int64)
    np.add.at(counts, (core, grp), 1)
    jrs = [-(-int(counts[:, g].max()) // P) for g in range(NG)]
    ntoks = [P * jr for jr in jrs]
    foff = np.cumsum([0] + ntoks).tolist()

    fb = np.asarray(feats_full, np.float32).astype(ml_dtypes.bfloat16)
    in_maps = []
    for k in range(NCORES):
        fa = np.zeros((foff[-1], C), ml_dtypes.bfloat16)
        ia = np.empty((16, foff[-1] // 16), np.int16)
        for g in range(NG):
            jr, ntok = jrs[g], ntoks[g]
            rows = np.nonzero((core == k) & (grp == g))[0]
            n = rows.size
            i = np.arange(n)
            fa[foff[g] + (i % P) * jr + i // P] = fb[rows]
            ip = np.arange(ntok - n)
            vals = np.empty(ntok, np.int16)
            vals[:n] = sid[rows].astype(np.int16)
            vals[n:] = _dump_slot(g) * P + ip % P
            ia[:, foff[g] // 16:foff[g + 1] // 16] = vals.reshape(ntok // 16, 16).T
        in_maps.append({
            "feats": fa,
            "sidx": np.ascontiguousarray(np.tile(ia, (8, 1))),
            "idin": np.eye(P, dtype=ml_dtypes.bfloat16),
        })
    return in_maps, tuple(jrs)


def kernel(batch_pillar_features, batch_indices, sample_indices, batch_size):
    global LAST_RESULTS
    feats_full = np.asarray(batch_pillar_features, np.float32)
    batch_indices = np.asarray(batch_indices)
    sample_indices = np.asarray(sample_indices)
    bs = int(batch_size)
    assert bs == B and feats_full.shape[1] == C

    in_maps, jrs = _prep_in_maps(feats_full, batch_indices, sample_indices)
    if _CACHE.get("jrs") != jrs:
        _CACHE["nc"] = _build_program(jrs)
        _CACHE["jrs"] = jrs
    nc = _CACHE["nc"]

    res = run_bass_kernel_spmd(nc, in_maps, core_ids=list(range(NCORES)))
    LAST_RESULTS = res

    full = np.empty((B, C, NX, NY), np.float32)
    for k in range(NCORES):
        b, hh = k // 2, k % 2
        r = np.asarray(res.results[k]["out"]).astype(np.float32).reshape(C, XH, NY)
        full[b, :, hh * XH:(hh + 1) * XH, :] = r
    return full
